# revision 1
# baseline (speedup 1.0000x reference)
"""Causal multi-head attention block (B=2, T=2048, C=1024, H=16) on 8 TRN2 cores.

Sharding: data-parallel over batch (2) x tensor-parallel over head groups (4).
core = 4*b + g handles batch b, heads [4g, 4g+4). Each core computes its
heads' attention output and a partial projection; the host sums the 4 partials
per batch and adds proj_b.

v2 schedule (vs baseline; TimelineSim 174432 -> 146722 ns):
- x / qkv weights land in bf16, out in bf16 (host upcasts); attention math
  stays f32r with fp32 PSUM accumulation (rel err 3.5e-3).
- DMA order: wqt/slab0 halves first (first Q matmul at ~4.3us instead of
  8.6us), then K/V weights, constants, pwt, and slabs prefetched 2 rows ahead.
- proj accumulates in psB [128,512] tiles, bounces PSUM->SBUF on DVE, and
  ships one merged [128,1024] bf16 DMA per t-tile (HWDGE slots are 625ns and
  serialize across queues - fewer, bigger stores win).
- proj(i) is deferred into row i+1 and drip-fed between attention visits as
  PE filler; Q(i+1) is emitted as 4 chunks in row i's diagonal phase (the
  visit cadence is ACT-exp-bound at ~1040ns vs 854ns of PE work per visit).
- pair-1 probs prefetch (NPRE1) + next-row pair-0 probs prefetch (NPRE0, PV
  deferred via the ppu ring) keep the exp pipeline ahead of PE's PV stream.
- the softmax reciprocal buffer ri33 is its own memset-initialized tile:
  writing it inside a DMA-loaded const tile put a wait-for-all-DMAs barrier
  at the head of the DVE stream (framework WAR aggregation).
- last row's projection is software-pipelined: yt0 start-matmuls issue before
  norm1, stop-matmuls/copies/DMAs drain 4-deep across psA+psB slots.
"""
import numpy as np
from contextlib import ExitStack

import concourse.bacc as bacc
import concourse.tile as tile
import concourse.mybir as mybir
from concourse.bass_utils import run_bass_kernel_spmd

F32 = mybir.dt.float32
F32R = mybir.dt.float32r
BF16 = mybir.dt.bfloat16
AF = mybir.ActivationFunctionType

T = 2048          # sequence length
C = 1024          # channels
HD = 64           # head dim
QW = 512          # q-tile width
NQI = T // QW     # 4 q-tiles
NKV = T // 128    # 16 kv-tiles
KC = C // 128     # 8 channel k-tiles
SCALE = HD ** -0.5

NPRE1 = 12         # pair-1 probs prefetched before the K/V slab block
NPRE0 = 12         # next-row pair-0 probs prefetched during pair-1 phase

_NC_CACHE = {}


def build_nc(repeat=1, bf16_in=True, use_bias=False, npre1=None, npre0=None, ppb=7, ppub=24, bigxb=3, sob=3, osbb=5):
    npre1 = NPRE1 if npre1 is None else npre1
    npre0 = NPRE0 if npre0 is None else npre0
    nc = bacc.Bacc("TRN2", target_bir_lowering=False)

    XDT = BF16 if bf16_in else F32R
    xt_d = nc.dram_tensor("xt", [C, T], XDT, kind="ExternalInput")
    wqt_d = nc.dram_tensor("wqt", [C, 256], XDT, kind="ExternalInput")
    wkt_d = nc.dram_tensor("wkt", [C, 256], XDT, kind="ExternalInput")
    wvt_d = nc.dram_tensor("wvt", [C, 256], XDT, kind="ExternalInput")
    # pwt pre-swizzled on host to [128, 2*C] so the DMA is fully contiguous
    pwt_d = nc.dram_tensor("pwt", [128, 2 * C], F32R, kind="ExternalInput")
    # constants: trib [128, 644] = tri(640) | onescol(4)
    # cb1 [128, 4] = qb(2) | kb(2)
    # cb2 [33, 512] = sel(128) | row0: ones1(128) | vb(256)
    trib_d = nc.dram_tensor("trib", [128, 644], BF16, kind="ExternalInput")
    cb1_d = nc.dram_tensor("cb1", [128, 4], F32R, kind="ExternalInput")
    cb2_d = nc.dram_tensor("cb2", [33, 512], F32R, kind="ExternalInput")
    out_d = nc.dram_tensor("out", [T, C], BF16, kind="ExternalOutput")

    with tile.TileContext(nc) as tc, ExitStack() as ctx:
        const = ctx.enter_context(tc.tile_pool(name="const", bufs=1))
        big = ctx.enter_context(tc.tile_pool(name="big", bufs=1))
        bigx = ctx.enter_context(tc.tile_pool(name="bigx", bufs=bigxb))
        pp = ctx.enter_context(tc.tile_pool(name="pp", bufs=ppb))
        ppu = ctx.enter_context(tc.tile_pool(name="ppu", bufs=ppub))
        pp2 = ctx.enter_context(tc.tile_pool(name="pp2", bufs=3))
        psA = ctx.enter_context(tc.tile_pool(name="psA", bufs=2, space="PSUM"))
        psB = ctx.enter_context(tc.tile_pool(name="psB", bufs=2, space="PSUM"))
        psO = ctx.enter_context(tc.tile_pool(name="psO", bufs=1, space="PSUM"))

        # ---- DMA prologue: slab0 + wqt first (Q matmuls gate startup) ----
        wqt_sb = const.tile([128, KC, 256], XDT, name="wqt")
        wkt_sb = const.tile([128, KC, 256], XDT, name="wkt")
        wvt_sb = const.tile([128, KC, 256], XDT, name="wvt")
        xt3 = xt_d.rearrange("(k p) t -> p k t", p=128)
        wq3 = wqt_d.rearrange("(k p) c -> p k c", p=128)
        xt_t = {}

        def emit_xt_slab(ts, first=False):
            slab = slab0 if first else bigx.tile([128, KC, QW], XDT, name="xt")
            for k in range(KC):
                xt_t[(k, ts)] = slab[:, k, :]
            if first:
                nc.sync.dma_start(wqt_sb[:, 0:2, :], wq3[:, 0:2, :])
                nc.sync.dma_start(slab[:, 0:2, :], xt3[:, 0:2, 0:QW])
                nc.sync.dma_start(wqt_sb[:, 2:4, :], wq3[:, 2:4, :])
                nc.sync.dma_start(slab[:, 2:4, :], xt3[:, 2:4, 0:QW])
                nc.sync.dma_start(slab[:, 4:6, :], xt3[:, 4:6, 0:QW])
                nc.sync.dma_start(wqt_sb[:, 4:8, :], wq3[:, 4:8, :])
                nc.sync.dma_start(slab[:, 6:8, :], xt3[:, 6:8, 0:QW])
                nc.sync.dma_start(cb1[:], cb1_d[:])
                nc.sync.dma_start(wkt_sb[:], wkt_d.rearrange("(k p) c -> p k c", p=128))
                nc.sync.dma_start(wvt_sb[:], wvt_d.rearrange("(k p) c -> p k c", p=128))
                return
            else:
                nc.sync.dma_start(slab[:], xt3[:, :, QW * ts:QW * (ts + 1)])

        slab0 = bigx.tile([128, KC, QW], XDT, name="xt")

        trib = const.tile([128, 644], BF16, name="trib")
        cb1 = const.tile([128, 4], F32R, name="cb1")
        cb2 = const.tile([33, 512], F32R, name="cb2")
        pwt_sb = const.tile([128, 2, C], F32R, name="pwt")
        emit_xt_slab(0, first=True)
        nc.sync.dma_start(trib[:], trib_d[:])
        nc.sync.dma_start(cb2[:], cb2_d[:])

        tri_sb = trib[:, 0:640]
        onescol_sb = trib[:, 640:644]
        qb_sb = cb1[:, 0:2].bitcast(F32)
        kb_sb = cb1[:, 2:4].bitcast(F32)
        sel_sb = cb2[:, 0:128]
        ones_sb = cb2[0:1, 128:256]
        vb_sb = cb2[0:1, 256:512]
        ri33_t = const.tile([33, 512], F32, name="ri33")
        # zero-init (no DMA dependency -> DVE stream head stays free;
        # rows 1..31 are multiplied by sel zeros and must not be NaN garbage)
        nc.vector.memset(ri33_t[:], 0.0)
        ri33 = ri33_t[:].bitcast(F32R)

        qt_t, kt_t, vn_t, yt_t = {}, {}, {}, {}

        def q_chunks(ts):
            # Q^T pair tiles for t-slab ts as 4 filler chunks (short psB
            # slot hold per chunk; first chunks only need the half-slab DMA)
            ps = {}

            def half1(p):
                ps[p] = psB.tile([128, QW], F32, name="acc")
                for k in range(4):
                    nc.tensor.matmul(ps[p][:], wqt_sb[:, k, 128 * p:128 * (p + 1)],
                                     xt_t[(k, ts)], start=(k == 0), stop=False)


            def half2(p):
                for k in range(4, KC):
                    nc.tensor.matmul(ps[p][:], wqt_sb[:, k, 128 * p:128 * (p + 1)],
                                     xt_t[(k, ts)], start=False, stop=(k == KC - 1))
                qt = big.tile([128, QW], BF16, name=f"qt_{p}_{ts}")
                nc.vector.tensor_scalar_add(qt[:], ps[p][:], qb_sb[:, p:p + 1])
                qt_t[(p, ts)] = qt

            return [lambda: half1(0), lambda: half1(1),
                    lambda: half2(0), lambda: half2(1)]

        def emit_q_slab(ts):
            for f in q_chunks(ts):
                f()

        def emit_k_slab(ts):
            for p in range(2):
                psk = psB.tile([128, QW], F32, name="acc")
                for k in range(KC):
                    nc.tensor.matmul(psk[:], wkt_sb[:, k, 128 * p:128 * (p + 1)],
                                     xt_t[(k, ts)], start=(k == 0), stop=(k == KC - 1))
                kt = big.tile([128, QW], BF16, name=f"kt_{p}_{ts}")
                nc.vector.tensor_scalar_add(kt[:], psk[:], kb_sb[:, p:p + 1])
                kt_t[(p, ts)] = kt

        def emit_v_tile(ti):
            # V natural tile (kv-tile ti), [128, 4*65] with ones cols
            ts = ti // 4
            psv = psB.tile([128, 256], F32, name="acc", tag="acc")
            for k in range(KC):
                nc.tensor.matmul(psv[:], xt_t[(k, ts)][:, 128 * (ti % 4):128 * (ti % 4 + 1)],
                                 wvt_sb[:, k, :], start=(k == 0),
                                 stop=(not use_bias and k == KC - 1))
            if use_bias:
                nc.tensor.matmul(psv[:], ones_sb[0:1, :], vb_sb[0:1, :],
                                 start=False, stop=True)
            vn = big.tile([128, 260], BF16, name=f"vn_{ti}")
            vn3 = vn[:].rearrange("a (h c) -> a h c", h=4, c=65)
            nc.vector.tensor_copy(vn3[:, :, 64:65], onescol_sb.rearrange("a (h c) -> a h c", h=4, c=1))
            nc.vector.tensor_copy(
                vn3[:, :, 0:64],
                psv[:].rearrange("a (h c) -> a h c", h=4, c=64))
            vn_t[ti] = vn

        def emit_probs(qi, p, kv, pool, tag):
            # logits (row-packed pair) + exp (+ triangle mask on diagonal tiles)
            o = 128 * kv - QW * qi
            full = o < 0
            o_pv = 0 if full else min(o, 384)
            kts = kt_t[(p, kv // 4)]
            kvs = slice(128 * (kv % 4), 128 * (kv % 4 + 1))
            qts = qt_t[(p, qi)]
            lp = psA.tile([128, 2 * QW], F32, name="lp")
            nc.tensor.matmul(lp[:, o_pv:QW], kts[0:64, kvs],
                             qts[0:64, o_pv:QW], start=True, stop=True)
            nc.tensor.matmul(lp[:, QW + o_pv:2 * QW], kts[64:128, kvs],
                             qts[64:128, o_pv:QW], start=True, stop=True)
            p_t = pool.tile([128, 2 * QW], BF16, name=tag, tag=tag)
            if o_pv == 0:
                nc.scalar.activation(p_t[:], lp[:], AF.Exp, scale=SCALE)
            else:
                seg = lambda ap, lo, hi: ap[:].rearrange(
                    "a (s q) -> a s q", s=2, q=QW)[:, :, lo:hi]
                nc.scalar.activation(seg(p_t, o_pv, QW), seg(lp, o_pv, QW),
                                     AF.Exp, scale=SCALE)
            if not full:
                # triangle mask on [o_pv, o+128): tri[kv, u], u = q - o + 512
                w = o + 128 - o_pv
                trs = tri_sb[:, 512 - (o - o_pv):640]
                sgm = p_t[:].rearrange("a (s q) -> a s q", s=2, q=QW)[:, :, o_pv:o_pv + w]
                trs2 = trs.rearrange("a (s b) -> a s b", s=1).broadcast_to([128, 2, w])
                nc.vector.tensor_mul(sgm, sgm, trs2)
            return p_t, o_pv

        def emit_pv(qi, p, o0, o1, kv, o_pv, p_t):
            nkv = 4 * (qi + 1)
            vn = vn_t[kv]
            nc.tensor.matmul(o0[0:65, o_pv:QW], vn[:, 130 * p:130 * p + 65],
                             p_t[:, o_pv:QW],
                             start=(kv == 0), stop=(kv == nkv - 1))
            nc.tensor.matmul(o1[0:65, o_pv:QW], vn[:, 130 * p + 65:130 * p + 130],
                             p_t[:, QW + o_pv:2 * QW],
                             start=(kv == 0), stop=(kv == nkv - 1))

        def emit_triple(qi, p, o0, o1, kv):
            p_t, o_pv = emit_probs(qi, p, kv, pp, "p_t")
            emit_pv(qi, p, o0, o1, kv, o_pv, p_t)

        def emit_norm(qi, p, o0, o1, so_on_act=False):
            with nc.allow_low_precision("f32r recip: 1e-4 rel is fine for softmax denom"):
                nc.vector.reciprocal(ri33[0:1, :], o0[64:65, :])
                nc.vector.reciprocal(ri33[32:33, :], o1[64:65, :])
            # copy O out of PSUM immediately so the accumulator banks free up
            # for the next pair; the normalize multiply then runs off-path
            # reading the SBUF copy x nb (single PSUM operand)
            so = pp2.tile([128, QW], F32R, name="so", bufs=sob)
            if so_on_act:
                nc.scalar.activation(so[0:64, :], o0[0:64, :], AF.Copy)
                nc.scalar.activation(so[64:128, :], o1[0:64, :], AF.Copy)
            else:
                nc.vector.tensor_copy(so[0:64, :], o0[0:64, :])
                nc.vector.tensor_copy(so[64:128, :], o1[0:64, :])
            nb = psB.tile([128, QW], F32, name="nb", tag="acc")
            nc.tensor.matmul(nb[:], sel_sb, ri33, start=True, stop=True)
            yt = big.tile([128, QW], F32R, name=f"yt_{p}_{qi}")
            nc.vector.tensor_mul(yt[:], so[:], nb[:])
            yt_t[(p, qi)] = yt

        def emit_proj_unit(qi, ti):
            # one [128t, 1024c] proj row-tile: per co-half 2 matmuls (ch
            # accumulate) + PSUM->SBUF copy; single merged DMA out
            tsl = slice(128 * (ti % 4), 128 * (ti % 4 + 1))
            osb = pp2.tile([128, 2 * QW], BF16, name="osb", bufs=osbb)
            for co in range(2):
                prj = psB.tile([128, QW], F32, name="prj", tag="acc")
                for ch in range(2):
                    nc.tensor.matmul(prj[:],
                                     yt_t[(ch, qi)][:, tsl],
                                     pwt_sb[:, ch, QW * co:QW * (co + 1)],
                                     start=(ch == 0), stop=(ch == 1))
                nc.vector.tensor_copy(osb[:, QW * co:QW * (co + 1)], prj[:])
            nc.sync.dma_start(out_d[128 * ti:128 * (ti + 1), :], osb[:])

        # ---- row loop with proj-deferral + filler interleave ----
        # slab DMAs are issued two rows ahead (bigx ring bufs=3) so the SP
        # queue has them in flight well before Q(i) needs the data
        n_slabs = repeat * NQI
        slab_cursor = [1]        # slab 0 already emitted (prologue)

        def prefetch_slabs(upto):
            while slab_cursor[0] <= min(upto, n_slabs - 1):
                emit_xt_slab(slab_cursor[0] % NQI)
                slab_cursor[0] += 1

        prefetch_slabs(1)
        nc.sync.dma_start(pwt_sb[:], pwt_d.rearrange("p (k c) -> p k c", k=2))
        for rep in range(repeat):
            fillers = []
            reserve = [0]

            def drain(n):
                for _ in range(n):
                    if len(fillers) > reserve[0]:
                        fillers.pop(0)()

            emit_q_slab(0)
            deferred0 = []       # next-row pair-0 probs prefetched a row early
            for i in range(NQI):
                g = rep * NQI + i
                reserve[0] = 2 if i == NQI - 1 else 0
                prefetch_slabs(g + 2)
                o0 = psO.tile([128, QW], F32, name="o0")
                o1 = psO.tile([128, QW], F32, name="o1")
                for kv, o_pv, p_t in deferred0:
                    emit_pv(i, 0, o0, o1, kv, o_pv, p_t)
                    drain(1)
                for kv in range(len(deferred0), 4 * i):
                    emit_triple(i, 0, o0, o1, kv)
                    drain(1)
                deferred0 = []
                # prefetch pair 1's first below-diagonal probs (PV deferred)
                # so ACT has work while PE runs the K/V slab matmuls
                npre = min(npre1, 4 * i)
                deferred = []
                for kv in range(npre):
                    deferred.append((kv,) + tuple(reversed(emit_probs(i, 1, kv, ppu, "p_u"))))
                emit_k_slab(i)
                for ti in range(4 * i, 4 * i + 4):
                    emit_v_tile(ti)
                    drain(1)
                qfill = q_chunks(i + 1) if i + 1 < NQI else []
                for kv in range(4 * i, 4 * (i + 1)):
                    emit_triple(i, 0, o0, o1, kv)
                    if qfill:
                        qfill.pop(0)()
                    else:
                        drain(1)
                for f in qfill:
                    f()
                emit_norm(i, 0, o0, o1)
                o0 = psO.tile([128, QW], F32, name="o0")
                o1 = psO.tile([128, QW], F32, name="o1")
                for kv, o_pv, p_t in deferred:
                    emit_pv(i, 1, o0, o1, kv, o_pv, p_t)
                    drain(1)
                for kv in range(npre, 4 * (i + 1)):
                    emit_triple(i, 1, o0, o1, kv)
                    drain(1)
                if i + 1 < NQI:
                    # next-row pair-0 probs: exp lead for row i+1 + PE work
                    # for the scheduler while this row's norm chain runs
                    for kv in range(min(npre0, 4 * (i + 1))):
                        deferred0.append(
                            (kv,) + tuple(reversed(emit_probs(i + 1, 0, kv, ppu, "p_u"))))
                emit_norm(i, 1, o0, o1, so_on_act=(i == NQI - 1))
                if i + 1 < NQI:
                    for ti in range(4 * i, 4 * i + 4):
                        fillers.append(lambda qi=i, ti=ti: emit_proj_unit(qi, ti))
            reserve[0] = 0
            # last row's proj: software-pipelined tail. start-mms (pair-0 yt)
            # can precede norm1; stop-mms + copies (alternating DVE/Pool) +
            # DMAs drain with <=2 units open on the psB ring.
            i = NQI - 1
            units = list(range(4 * i, 4 * i + 4))
            open_prj = []

            def unit_start(k):
                ti = units[k]
                for co in range(2):
                    pool_, tag_ = (psB, "acc") if co else (psA, "lp")
                    prj = pool_.tile([128, QW], F32, name="prj", tag=tag_)
                    nc.tensor.matmul(prj[:], yt_t[(0, i)][:, 128 * (ti % 4):128 * (ti % 4 + 1)],
                                     pwt_sb[:, 0, QW * co:QW * (co + 1)],
                                     start=True, stop=False)
                    open_prj.append(prj)

            def unit_finish(k):
                ti = units[k]
                osb = pp2.tile([128, 2 * QW], BF16, name="osb", bufs=osbb)
                for co in range(2):
                    prj = open_prj.pop(0)
                    nc.tensor.matmul(prj[:], yt_t[(1, i)][:, 128 * (ti % 4):128 * (ti % 4 + 1)],
                                     pwt_sb[:, 1, QW * co:QW * (co + 1)],
                                     start=False, stop=True)
                    if (2 * k + co) % 2:
                        nc.vector.tensor_copy(osb[:, QW * co:QW * (co + 1)], prj[:])
                    else:
                        nc.scalar.activation(osb[:, QW * co:QW * (co + 1)], prj[:], AF.Copy)
                dq = nc.sync if k % 2 else nc.scalar
                dq.dma_start(out_d[128 * ti:128 * (ti + 1), :], osb[:])

            drain(len(fillers))
            unit_start(0)
            unit_start(1)
            for k in range(len(units)):
                unit_finish(k)
                if k + 2 < len(units):
                    unit_start(k + 2)

    nc.compile()
    return nc


def make_in_maps(x, qkv_w, qkv_b, proj_w, bf16_in=True):
    """Shard inputs for 8 cores: core = 4*b + g."""
    tri = (np.arange(640)[None, :] >= (np.arange(128)[:, None] + 512)).astype(np.float32)
    sel = np.zeros((33, 128), np.float32)
    sel[0, 0:64] = 1.0
    sel[32, 64:128] = 1.0
    in_maps = []
    for core in range(8):
        b, g = core // 4, core % 4
        r0 = 4 * g * HD          # first q/k/v row of this head group (256 rows)
        pw = np.ascontiguousarray(proj_w[:, r0:r0 + 256].T)   # [256, C]
        pw = pw.reshape(2, 128, C).transpose(1, 0, 2).reshape(128, 2 * C)
        m = {
            "xt": np.ascontiguousarray(x[b].T),
            "wqt": np.ascontiguousarray(qkv_w[r0:r0 + 256, :].T),
            "wkt": np.ascontiguousarray(qkv_w[C + r0:C + r0 + 256, :].T),
            "wvt": np.ascontiguousarray(qkv_w[2 * C + r0:2 * C + r0 + 256, :].T),
            "pwt": np.ascontiguousarray(pw),
            "qb": np.ascontiguousarray(qkv_b[r0:r0 + 256].reshape(2, 128).T),
            "kb": np.ascontiguousarray(qkv_b[C + r0:C + r0 + 256].reshape(2, 128).T),
            "vb": qkv_b[2 * C + r0:2 * C + r0 + 256].reshape(1, 256).copy(),
        }
        trib = np.zeros((128, 644), np.float32)
        trib[:, 0:640] = tri
        trib[:, 640:644] = 1.0
        cb1 = np.zeros((128, 4), np.float32)
        cb1[:, 0:2] = m.pop("qb")
        cb1[:, 2:4] = m.pop("kb")
        m["trib"] = trib
        cb2 = np.zeros((33, 512), np.float32)
        cb2[:, 0:128] = sel
        cb2[0, 128:256] = 1.0
        cb2[0:1, 256:512] = m.pop("vb")
        m["cb1"] = cb1
        m["cb2"] = cb2
        import ml_dtypes
        bf16_keys = ({"xt", "wqt", "wkt", "wvt"} if bf16_in else set()) | {"trib"}
        in_maps.append({
            k: np.ascontiguousarray(v, dtype=(ml_dtypes.bfloat16 if k in bf16_keys else np.float32))
            for k, v in m.items()})
    return in_maps


def kernel(x, qkv_w, qkv_b, proj_w, proj_b):
    x = np.asarray(x, dtype=np.float32)
    qkv_w = np.asarray(qkv_w, dtype=np.float32)
    qkv_b = np.asarray(qkv_b, dtype=np.float32)
    proj_w = np.asarray(proj_w, dtype=np.float32)
    proj_b = np.asarray(proj_b, dtype=np.float32)

    use_bias = bool(np.any(qkv_b))
    key = ("nc", use_bias)
    if key not in _NC_CACHE:
        _NC_CACHE[key] = build_nc(use_bias=use_bias)
    nc = _NC_CACHE[key]
    _NC_CACHE["nc"] = nc
    in_maps = make_in_maps(x, qkv_w, qkv_b, proj_w)
    res = run_bass_kernel_spmd(nc, in_maps, core_ids=list(range(8)))
    out = np.zeros((2, T, C), np.float32)
    for core in range(8):
        out[core // 4] += np.asarray(res.results[core]["out"], dtype=np.float32)
    out += proj_b[None, None, :]
    return out

